# revision 18
# baseline (speedup 1.0000x reference)
"""Bass/Trainium2 kernel for nn_BidirectionalAgg (hyperbolic GNN bidirectional
aggregation): out = proj(expmap0(att_chi @ x_t + att_par @ x_t)) where
att_par = adj * sigmoid(sl_p[i] + sr_p[j] + b_p), att_chi = adj.T * sigmoid(...),
x_t = logmap0(x).

Sharding: 8 NeuronCores, core k owns output rows [1024k, 1024k+1024).
Each core receives:
  m_par [8192, 1024] fp16 : adj[blk, :].T  (column-block of adj.T), row-rotated
  m_chi [8192, 1024] fp16 : adj[:, blk],                           row-rotated
  xf    [8192, 128] fp32  : x, row-rotated so the core's own rows come first
  w4    [128, 4]    fp32  : [w_par[:d], w_par[d:], w_chi[:d], w_chi[d:]]
  bb    [1, 2]      fp32  : [b_par, b_chi]
  id16/id32               : identity matrices for TensorE transposes
The row rotation makes the SPMD program identical on every core (its own
block is always j-tiles 0..7). The j-contraction is permutation invariant.
"""

import os
import sys

sys.path.insert(0, "/opt/trn_rl_repo")

import numpy as np

N = 8192
D = 128
NCORES = 8
B = N // NCORES          # 1024 rows per core
T = N // 128             # 64 j-tiles
TB = B // 128            # 8 tiles in own block

KMODE = os.environ.get("KMODE", "full")   # full | p12 | p34  (debug bisection)

_CACHE = {}
LAST_RESULTS = None


def _build():
    import concourse.bacc as bacc
    import concourse.mybir as mybir
    import concourse.tile as tile
    from concourse.bass import MemorySpace

    dt = mybir.dt
    AF = mybir.ActivationFunctionType
    ALU = mybir.AluOpType
    do12 = KMODE in ("full", "p12")
    do34 = KMODE in ("full", "p34")

    nc = bacc.Bacc("TRN2", target_bir_lowering=False, debug=False,
                   num_devices=NCORES)

    m_par = nc.dram_tensor("m_par", [N, B], dt.float16, kind="ExternalInput")
    m_chi = nc.dram_tensor("m_chi", [N, B], dt.float16, kind="ExternalInput")
    xf = nc.dram_tensor("xf", [N, D], dt.float32, kind="ExternalInput")
    w4 = nc.dram_tensor("w4", [D, 4], dt.float32, kind="ExternalInput")
    bb = nc.dram_tensor("bb", [1, 2], dt.float32, kind="ExternalInput")
    id16 = nc.dram_tensor("id16", [128, 128], dt.float16, kind="ExternalInput")
    id32 = nc.dram_tensor("id32", [128, 128], dt.float32, kind="ExternalInput")
    out = nc.dram_tensor("out", [B, D], dt.float32, kind="ExternalOutput")

    with tile.TileContext(nc) as tc:
        with (
            tc.tile_pool(name="const", bufs=1) as const,
            tc.tile_pool(name="big", bufs=1) as big,
            tc.tile_pool(name="work", bufs=3) as work,
            tc.tile_pool(name="mstream", bufs=4) as mstream,
            tc.tile_pool(name="psum", bufs=2, space=MemorySpace.PSUM) as pp,
            tc.tile_pool(name="psacc", bufs=1, space=MemorySpace.PSUM) as pacc,
        ):
            ident16 = const.tile([128, 128], dt.float16)
            nc.sync.dma_start(ident16[:], id16.ap())
            ident32 = const.tile([128, 128], dt.float32)
            nc.sync.dma_start(ident32[:], id32.ap())
            ones1 = const.tile([1, 128], dt.float32)
            nc.vector.memset(ones1[:], 1.0)

            w4s = const.tile([D, 4], dt.float32)
            nc.sync.dma_start(w4s[:], w4.ap())
            w4h = const.tile([D, 4], dt.float16)
            nc.vector.tensor_copy(w4h[:], w4s[:])

            bbs = const.tile([1, 2], dt.float32)
            nc.sync.dma_start(bbs[:], bb.ap())
            psb = pp.tile([128, 2], dt.float32, tag="ps")
            nc.tensor.matmul(psb[:], ones1[:], bbs[:], start=True, stop=True)
            bpbc = const.tile([128, 2], dt.float32)
            nc.scalar.copy(bpbc[:], psb[:])
            bp_b = bpbc[:, 0:1]
            bc_b = bpbc[:, 1:2]

            xt16 = big.tile([128, T * D], dt.float16)       # x_t [j, (t d)]
            S = big.tile([128, T * 4], dt.float32)          # [j, (t v)]
            bcast_sl = []
            for ci in range(2):
                bcast_sl.append(big.tile([128, B], dt.float32,
                                         name=f"bcast{ci}",
                                         tag=f"bcast{ci}"))

            if not do12:
                nc.vector.memset(xt16[:], 0.01)
                nc.vector.memset(S[:], 0.0)
                nc.vector.memset(bcast_sl[0][:], 0.0)
                nc.vector.memset(bcast_sl[1][:], 0.0)

            if do12:
                # ------------ phase 1: load x, logmap0 -> x_t (fp16) -------
                xall = big.tile([128, T * D], dt.float32)   # x tiles [j, (t d)]
                n2 = big.tile([128, T], dt.float32)
                for t in range(T):
                    nc.sync.dma_start(xall[:, t * D:(t + 1) * D],
                                      xf.ap()[t * 128:(t + 1) * 128, :])
                    tr = work.tile([128, D], dt.float32, tag="trash")
                    nc.vector.tensor_mul(tr[:], xall[:, t * D:(t + 1) * D],
                                         xall[:, t * D:(t + 1) * D])
                    nc.vector.reduce_sum(n2[:, t:t + 1], tr[:],
                                         axis=mybir.AxisListType.X)

                # factor f = artanh(clip(norm)) / norm   (c = 1)
                u = big.tile([128, T], dt.float32)
                nc.scalar.activation(u[:], n2[:], AF.Sqrt)
                nc.vector.tensor_scalar_max(u[:], u[:], 1e-15)
                nc.vector.tensor_scalar_min(u[:], u[:], 1.0 - 1e-7)
                num = work.tile([128, T], dt.float32, tag="ftmp")
                nc.vector.tensor_scalar_add(num[:], u[:], 1.0)
                den = work.tile([128, T], dt.float32, tag="ftmp")
                nc.vector.tensor_scalar(den[:], u[:], -1.0, 1.0, ALU.mult,
                                        ALU.add)
                rden = work.tile([128, T], dt.float32, tag="ftmp")
                nc.vector.reciprocal(rden[:], den[:])
                rat = work.tile([128, T], dt.float32, tag="ftmp")
                nc.vector.tensor_mul(rat[:], num[:], rden[:])
                lg = work.tile([128, T], dt.float32, tag="ftmp")
                nc.scalar.activation(lg[:], rat[:], AF.Ln)
                ru = work.tile([128, T], dt.float32, tag="ftmp")
                nc.vector.reciprocal(ru[:], u[:])
                f = big.tile([128, T], dt.float32)
                nc.vector.scalar_tensor_tensor(out=f[:], in0=lg[:],
                                               scalar=0.5, in1=ru[:],
                                               op0=ALU.mult, op1=ALU.mult)

                for t in range(T):
                    nc.vector.tensor_scalar_mul(xt16[:, t * D:(t + 1) * D],
                                                xall[:, t * D:(t + 1) * D],
                                                f[:, t:t + 1])

                # ------------ phase 2: x_t^T, score vectors S --------------
                xtT = big.tile([128, T * 128], dt.float16)  # [d, (t j)]
                for t in range(T):
                    pt = pp.tile([128, 128], dt.float16, tag="ptr")
                    nc.tensor.transpose(pt[:], xt16[:, t * D:(t + 1) * D],
                                        ident16[:])
                    nc.vector.tensor_copy(xtT[:, t * 128:(t + 1) * 128],
                                          pt[:])
                    ps = pp.tile([128, 4], dt.float32, tag="ps")
                    nc.tensor.matmul(ps[:], xtT[:, t * 128:(t + 1) * 128],
                                     w4h[:], start=True, stop=True)
                    nc.scalar.copy(S[:, 4 * t:4 * t + 4], ps[:])

                S3 = S[:].rearrange("p (t v) -> p t v", v=4)
                nc.vector.tensor_scalar_add(S3[:, :, 1:2], S3[:, :, 1:2],
                                            bp_b)
                nc.vector.tensor_scalar_add(S3[:, :, 3:4], S3[:, :, 3:4],
                                            bc_b)

                # broadcast sl (own-block left scores) along the free dim
                for ci, c0 in enumerate((0, 2)):
                    pk = pp.tile([8, 128], dt.float32, tag="ps")
                    nc.tensor.transpose(pk[:], S3[:, 0:TB, c0:c0 + 1],
                                        ident32[:])
                    slrow = work.tile([8, 128], dt.float32, tag="slrow")
                    nc.scalar.copy(slrow[:], pk[:])
                    bc = bcast_sl[ci]
                    for r in range(TB):
                        # broadcast row r to all 128 partitions via a K=1
                        # matmul against a ones column (no GPSIMD ucode).
                        stage = work.tile([1, 128], dt.float32, tag="slstage")
                        nc.sync.dma_start(stage[:], slrow[r:r + 1, :])
                        pb = pp.tile([128, 128], dt.float32, tag="pbc")
                        nc.tensor.matmul(pb[:], ones1[:], stage[:],
                                         start=True, stop=True)
                        nc.scalar.copy(bc[:, r * 128:(r + 1) * 128], pb[:])

            if not do34:
                # debug output: dump bcast_sl + x_t tile so p12 is testable
                ot = work.tile([128, D], dt.float32, tag="ot")
                for r in range(TB):
                    src = bcast_sl[r % 2]
                    nc.vector.tensor_copy(
                        ot[:], src[:, (r // 2) * 128:(r // 2) * 128 + D])
                    nc.sync.dma_start(out.ap()[r * 128:(r + 1) * 128, :],
                                      ot[:])
            else:
                # ------------ phase 3: masked attention + matmul -----------
                acc = pacc.tile([128, B], dt.float32)       # [d, i'] 2 banks
                for term in range(2):
                    M = m_par if term == 0 else m_chi
                    bc = bcast_sl[term]
                    bias_c = 1 if term == 0 else 3
                    for t in range(T):
                        mt = mstream.tile([128, B], dt.float16, tag="mt")
                        nc.sync.dma_start(mt[:],
                                          M.ap()[t * 128:(t + 1) * 128, :])
                        sg = mstream.tile([128, B], dt.float16, tag="sg")
                        nc.scalar.activation(sg[:], bc[:], AF.Sigmoid,
                                             bias=S[:, 4 * t + bias_c:
                                                    4 * t + bias_c + 1])
                        mk = mstream.tile([128, B], dt.float16, tag="mk")
                        nc.vector.tensor_mul(mk[:], mt[:], sg[:])
                        # PSUM write per matmul is capped at one bank
                        # (512 fp32): split the 1024-wide update in two.
                        for hh in range(2):
                            nc.tensor.matmul(
                                acc[:, hh * 512:(hh + 1) * 512],
                                xt16[:, t * D:(t + 1) * D],
                                mk[:, hh * 512:(hh + 1) * 512],
                                start=(term == 0 and t == 0),
                                stop=(term == 1 and t == T - 1))

                # ------------ phase 4: expmap0 + proj + store --------------
                supT = big.tile([128, B], dt.float32)
                nc.scalar.copy(supT[:], acc[:])
                supN = big.tile([128, TB * D], dt.float32)  # [i, (r d)]
                n2o = work.tile([128, TB], dt.float32, tag="n2o")
                for r in range(TB):
                    pr = pp.tile([128, 128], dt.float32, tag="ptr")
                    nc.tensor.transpose(pr[:],
                                        supT[:, r * 128:(r + 1) * 128],
                                        ident32[:])
                    nc.vector.tensor_copy(supN[:, r * D:(r + 1) * D], pr[:])
                    tr = work.tile([128, D], dt.float32, tag="trash")
                    nc.vector.tensor_mul(tr[:], supN[:, r * D:(r + 1) * D],
                                         supN[:, r * D:(r + 1) * D])
                    nc.vector.reduce_sum(n2o[:, r:r + 1], tr[:],
                                         axis=mybir.AxisListType.X)

                u2 = work.tile([128, TB], dt.float32, tag="f2")
                nc.scalar.activation(u2[:], n2o[:], AF.Sqrt)
                nc.vector.tensor_scalar_max(u2[:], u2[:], 1e-15)
                th = work.tile([128, TB], dt.float32, tag="f2")
                nc.scalar.activation(th[:], u2[:], AF.Tanh)
                ru2 = work.tile([128, TB], dt.float32, tag="f2")
                nc.vector.reciprocal(ru2[:], u2[:])
                g = work.tile([128, TB], dt.float32, tag="f2")
                nc.vector.tensor_mul(g[:], th[:], ru2[:])
                thc = work.tile([128, TB], dt.float32, tag="f2")
                nc.vector.tensor_scalar_max(thc[:], th[:], 1e-7)
                rny = work.tile([128, TB], dt.float32, tag="f2")
                nc.vector.reciprocal(rny[:], thc[:])
                cap = work.tile([128, TB], dt.float32, tag="f2")
                nc.vector.tensor_scalar(cap[:], rny[:], 1.0 - 1e-5, 1.0,
                                        ALU.mult, ALU.min)
                h = work.tile([128, TB], dt.float32, tag="f2")
                nc.vector.tensor_mul(h[:], g[:], cap[:])

                for r in range(TB):
                    ot = work.tile([128, D], dt.float32, tag="ot")
                    nc.vector.tensor_scalar_mul(ot[:],
                                                supN[:, r * D:(r + 1) * D],
                                                h[:, r:r + 1])
                    nc.sync.dma_start(out.ap()[r * 128:(r + 1) * 128, :],
                                      ot[:])

    nc.compile()
    return nc


def _get_nc():
    if "nc" not in _CACHE:
        _CACHE["nc"] = _build()
    return _CACHE["nc"]


def _in_maps(x, adj16, w4, bb, id16, id32):
    maps = []
    for k in range(NCORES):
        lo, hi = k * B, (k + 1) * B
        mp = np.roll(adj16[lo:hi, :].T, -lo, axis=0)
        mc = np.roll(adj16[:, lo:hi], -lo, axis=0)
        xk = np.roll(x, -lo, axis=0)
        maps.append({
            "m_par": np.ascontiguousarray(mp),
            "m_chi": np.ascontiguousarray(mc),
            "xf": np.ascontiguousarray(xk),
            "w4": w4,
            "bb": bb,
            "id16": id16,
            "id32": id32,
        })
    return maps


def kernel(x, adj, w_par, b_par, w_chi, b_chi):
    global LAST_RESULTS
    from concourse.bass_utils import run_bass_kernel_spmd

    x = np.asarray(x, np.float32)
    adj16 = np.asarray(adj).astype(np.float16)      # 0/1 entries: exact
    w_par = np.asarray(w_par, np.float32)
    w_chi = np.asarray(w_chi, np.float32)
    w4 = np.stack([w_par[:D], w_par[D:], w_chi[:D], w_chi[D:]],
                  axis=1).astype(np.float32)
    bb = np.array([[np.float32(b_par[0]), np.float32(b_chi[0])]], np.float32)
    id16 = np.eye(128, dtype=np.float16)
    id32 = np.eye(128, dtype=np.float32)

    nc = _get_nc()
    res = run_bass_kernel_spmd(nc, _in_maps(x, adj16, w4, bb, id16, id32),
                               list(range(NCORES)))
    LAST_RESULTS = res
    return np.concatenate([res.results[k]["out"] for k in range(NCORES)],
                          axis=0)
